# revision 11
# baseline (speedup 1.0000x reference)
"""Block-circulant linear (MINI_BLOCK=4) via length-4 rFFT factorization on 8 trn2 cores.

Math: out = x @ W^T where W[4y+n, 4x+j] = eigens[y, x, (n-j) mod 4].
In the length-4 DFT domain the circulant contraction factors into 5 real
matmul chains over the block-index axis gx=1024 (Gauss 3-mult for the complex
bin):
  X0 = x0+x1+x2+x3, X1 = (x0-x2) + i(x3-x1), X2 = x0-x1+x2-x3  (per block of 4)
  g1 = (X1r+X1i)E1r, g2 = X1r(E1i-E1r), g3 = X1i(E1r+E1i)
  Y0 = X0 E0, Y2 = X2 E2, Y1r = g1-g3, Y1i = g1+g2
  o0 = Y0+Y1r+Y2, o1 = Y0-Y1i-Y2, o2 = Y0-Y1r+Y2, o3 = Y0+Y1i-Y2  (scales in E)

Sharding: data-parallel over batch, 512 rows per core; E replicated.

Schedule (v2, HAM/DMA-aware):
- The PE clock-gate (HAM) keeps the array at 1.2 GHz until ~3.4us of
  sustained activity; idle gaps re-throttle it. A block of dummy matmuls on
  a zeroed SBUF tile wakes the PE right after the NEFF preamble so the array
  is warm when real data lands, and pass 0 trails the input stream at
  xc-slice granularity so the PE never takes one long stall.
- Inputs ship 4-bin-packed ([p, xc, bin, cols]) so each DMA call moves
  1-2 MB with 4-16 KB contiguous per-partition lines (14 input calls total
  on the SP ring, issued in exact consumption order). Derived bins
  (Ed=E1i-E1r, Es=E1i+E1r, X1s=X1r+X1i) are computed on device, halves
  split so the derive never gates a pass start.
- Per (128-wide y-chunk c, 256-wide batch half h) the five bins accumulate
  ps[k] = sum_xc E_k[c][:,xc,:].T @ X_k[h][:,xc,:]  -> [128 y, 256 b] PSUM
  (8 matmuls of 128x128x256 bf16 per bin), then ACT drains y0/g1, DVE
  combines the inverse DFT in bf16 and stores two [128,2,256] groups
  (n in {0,2} then {1,3}) per pass on the ACT ring so the tail store
  overlaps the drain chain. The last pass staggers bin completion
  (y0,g1,y2 xc-major, then g3, then g2) so its drains overlap its matmuls.
- PSUM: one bank per bin accumulator, bufs y0/g1:1 (freed first by the ACT
  copies), g2/y2/g3:2 = 8 banks.
Host de-interleaves [h,c,grp,y,pair,b] -> [256h+b, 4*(128c+y)+n].
"""
import numpy as np

B, IN, OUT, BLK = 4096, 4096, 4096, 4
GX, GY = IN // BLK, OUT // BLK        # 1024, 1024
NCORES = 8
BS = B // NCORES                      # 512 batch rows per core
XC = GX // 128                        # 8 x-chunks (contraction)
YC = GY // 128                        # 8 y-chunks (output)
BINS = ("y0", "g2", "y2", "g3", "g1")
PSUM_BUFS = {"y0": 1, "g1": 1, "g2": 2, "y2": 2, "g3": 2}
# PE warm-up matmuls: span preamble-end (~7.6us) to first X half landing
# (~14.2us): ~16 cold at 213ns (HAM un-throttles after ~3.4us) + ~28 warm.
NDUMMY = 44

_cache = {}


def _build_nc():
    from concourse import bacc
    import concourse.mybir as mybir
    from concourse.tile import TileContext

    f32 = mybir.dt.float32
    bf16 = mybir.dt.bfloat16

    nc = bacc.Bacc("TRN2", target_bir_lowering=False, debug=False,
                   enable_asserts=False, num_devices=NCORES)
    # E packed per chunk: [yc, p, xc, ki, y] with ki = (E0, E1r, E1i, E2).
    # 8KB contiguous per partition per chunk -> one call per chunk.
    e_pk = nc.dram_tensor("e_pk", [YC, 128, XC, 4, 128], bf16,
                          kind="ExternalInput")
    # X packed per half: [h, p, xc, ki, b] with ki = (X0, X1r, X2, X1i).
    # 16KB lines; half 0 ships in 2-xc slices for pass-0 trailing.
    x_pk = nc.dram_tensor("x_pk", [2, 128, XC, 4, 256], bf16,
                          kind="ExternalInput")
    # out [h, yc, grp, y, pair, b]: grp0 pairs (n0,n2), grp1 pairs (n1,n3).
    od_d = nc.dram_tensor("out", [2, YC, 2, 128, 2, 256], bf16,
                          kind="ExternalOutput")

    with TileContext(nc) as tc:
        with (
            tc.tile_pool(name="xm", bufs=1) as xp,
            tc.tile_pool(name="es", bufs=1) as ep,
            tc.tile_pool(name="tv", bufs=2) as tvp,
            tc.tile_pool(name="abcd", bufs=3) as ab,
            tc.tile_pool(name="otp", bufs=6) as op_,
            tc.tile_pool(name="ps", bufs=1, space="PSUM") as mps,
        ):
            # Persistent SBUF residency: X 32KB/part + 8KB derived,
            # E 64KB/part + 32KB derived.
            xm = [xp.tile([128, XC, 4, 256], bf16, tag=f"xm{h}", name=f"xm{h}")
                  for h in range(2)]
            xg = [xp.tile([128, XC, 256], bf16, tag=f"xg{h}", name=f"xg{h}")
                  for h in range(2)]
            es_pk = [ep.tile([128, XC, 4, 128], bf16, tag=f"es{c}",
                             name=f"es{c}") for c in range(YC)]
            ed_t = [ep.tile([128, XC, 128], bf16, tag=f"ed{c}", name=f"ed{c}")
                    for c in range(YC)]
            es_t = [ep.tile([128, XC, 128], bf16, tag=f"eds{c}",
                            name=f"eds{c}") for c in range(YC)]
            jk = xp.tile([128, 384], bf16, tag="jk", name="jk")

            # Input DMA on the SP ring in exact consumption order; the queue
            # drains FIFO so issue order == arrival order. The 16 SDMA
            # engines cap at ~26 GB/s each, so big descriptors (8-16KB) and
            # few call boundaries maximize the ~420 GB/s aggregate.
            nc.sync.dma_start(out=es_pk[0], in_=e_pk[0])
            for q in range(2):
                s = slice(4 * q, 4 * q + 4)
                nc.sync.dma_start(out=xm[0][:, s], in_=x_pk[0][:, s])
            nc.sync.dma_start(out=es_pk[1], in_=e_pk[1])
            for c in range(2, YC):
                nc.sync.dma_start(out=es_pk[c], in_=e_pk[c])
            nc.sync.dma_start(out=xm[1], in_=x_pk[1])

            # PE warm-up: zero a junk tile, then back-to-back dummy matmuls
            # into pass-0's y0 bank (its real group start=True resets it).
            nc.gpsimd.memset(jk, 0)

            def make_ps():
                return {k: mps.tile([128, 256], f32, tag=f"ps_{k}",
                                    name=f"ps_{k}", bufs=PSUM_BUFS[k])
                        for k in BINS}

            ps0 = make_ps()
            for _ in range(NDUMMY):
                nc.tensor.matmul(ps0["y0"], jk[:, 0:128], jk[:, 128:384],
                                 start=True, stop=True, skip_group_check=True)

            # Derived bins; chunk-0 and chunk-1 derives split by xc-half so
            # they trail the (split) DMA calls without gating pass starts.
            def derive_E(c, lo=0, hi=XC):
                s = slice(lo, hi)
                nc.vector.tensor_sub(out=ed_t[c][:, s], in0=es_pk[c][:, s, 2],
                                     in1=es_pk[c][:, s, 1])
                nc.vector.tensor_add(out=es_t[c][:, s], in0=es_pk[c][:, s, 2],
                                     in1=es_pk[c][:, s, 1])

            def derive_g1(h, lo=0, hi=XC):
                s = slice(lo, hi)
                nc.vector.tensor_add(out=xg[h][:, s], in0=xm[h][:, s, 1],
                                     in1=xm[h][:, s, 3])

            derive_E(0, 0, 4)
            derive_E(0, 4, 8)
            derive_g1(0, 0, 4)
            derive_g1(0, 4, 8)

            # bin -> (stationary source, moving ki slot); g2/g3 read derived
            # E tiles, g1 reads the derived X tile.
            def stat(k, c, xc):
                if k == "y0":
                    return es_pk[c][:, xc, 0]
                if k == "g1":
                    return es_pk[c][:, xc, 1]
                if k == "y2":
                    return es_pk[c][:, xc, 3]
                if k == "g2":
                    return ed_t[c][:, xc]
                return es_t[c][:, xc]

            def mov(k, h, xc):
                ki = {"y0": 0, "g2": 1, "y2": 2, "g3": 3}.get(k)
                return xg[h][:, xc] if ki is None else xm[h][:, xc, ki]

            def mm(ps, c, xc, k, h):
                nc.tensor.matmul(ps[k], stat(k, c, xc), mov(k, h, xc),
                                 start=xc == 0, stop=xc == XC - 1,
                                 skip_group_check=True)

            def drains(ps, c, h):
                # Inverse DFT: ACT drains g1/y0 (frees their single banks
                # fast); DVE reads one PSUM operand per op, combines in bf16.
                # Two stores per pass so the last store never serializes the
                # whole drain chain.
                v_ = tvp.tile([128, 256], f32, tag="v", name="v_")
                t_ = tvp.tile([128, 256], f32, tag="t", name="t_")
                a_ = ab.tile([128, 256], bf16, tag="a", name="a_")
                b_ = ab.tile([128, 256], bf16, tag="b", name="b_")
                c_ = ab.tile([128, 256], bf16, tag="c", name="c_")
                d_ = ab.tile([128, 256], bf16, tag="d", name="d_")
                o02 = op_.tile([128, 2, 256], bf16, tag="o02", name="o02")
                o13 = op_.tile([128, 2, 256], bf16, tag="o13", name="o13")
                nc.scalar.copy(out=t_, in_=ps["y0"])              # frees y0
                nc.scalar.copy(out=v_, in_=ps["g1"])              # frees g1
                nc.vector.tensor_add(out=a_, in0=t_, in1=ps["y2"])  # Y0+Y2
                nc.vector.tensor_sub(out=b_, in0=t_, in1=ps["y2"])  # Y0-Y2
                nc.vector.tensor_sub(out=c_, in0=v_, in1=ps["g3"])  # Y1r
                nc.vector.tensor_add(out=o02[:, 0], in0=a_, in1=c_)
                nc.vector.tensor_sub(out=o02[:, 1], in0=a_, in1=c_)
                nc.scalar.dma_start(out=od_d[h][c][0], in_=o02)
                nc.vector.tensor_add(out=d_, in0=v_, in1=ps["g2"])  # Y1i
                nc.vector.tensor_sub(out=o13[:, 0], in0=b_, in1=d_)
                nc.vector.tensor_add(out=o13[:, 1], in0=b_, in1=d_)
                nc.scalar.dma_start(out=od_d[h][c][1], in_=o13)

            for h in range(2):
                for c in range(YC):
                    ps = ps0 if (h == 0 and c == 0) else make_ps()
                    if not (h == 1 and c == YC - 1):
                        for xc in range(XC):
                            for k in BINS:
                                mm(ps, c, xc, k, h)
                        if h == 0 and c == 0:
                            # Pass-1 needs ed/es for chunk 1 right away;
                            # the first half lands before the DVE reaches it.
                            derive_E(1, 0, 4)
                        drains(ps, c, h)
                        # Derives AFTER the drains: a late chunk landing can
                        # then never head-block the drain chain in the DVE
                        # FIFO (which backs up into PSUM-bank PE stalls).
                        if h == 0 and c == 0:
                            derive_E(1, 4, 8)
                        elif h == 0 and c < YC - 1:
                            derive_E(c + 1)
                        if h == 0 and c == 5:
                            # X half-1 lands ~38us, well before pass 8.
                            derive_g1(1, 0, 4)
                            derive_g1(1, 4, 8)
                        continue
                    # Tail pass: stagger bin completion so the drains and
                    # first store overlap this pass's own matmuls.
                    for xc in range(XC):
                        for k in ("y0", "g1", "y2"):
                            mm(ps, c, xc, k, h)
                    v_ = tvp.tile([128, 256], f32, tag="v", name="v_")
                    t_ = tvp.tile([128, 256], f32, tag="t", name="t_")
                    a_ = ab.tile([128, 256], bf16, tag="a", name="a_")
                    b_ = ab.tile([128, 256], bf16, tag="b", name="b_")
                    c_ = ab.tile([128, 256], bf16, tag="c", name="c_")
                    d_ = ab.tile([128, 256], bf16, tag="d", name="d_")
                    o02 = op_.tile([128, 2, 256], bf16, tag="o02", name="o02")
                    o13 = op_.tile([128, 2, 256], bf16, tag="o13", name="o13")
                    nc.scalar.copy(out=t_, in_=ps["y0"])
                    nc.scalar.copy(out=v_, in_=ps["g1"])
                    nc.vector.tensor_add(out=a_, in0=t_, in1=ps["y2"])
                    nc.vector.tensor_sub(out=b_, in0=t_, in1=ps["y2"])
                    for xc in range(XC):
                        mm(ps, c, xc, "g2", h)
                    nc.vector.tensor_add(out=d_, in0=v_, in1=ps["g2"])
                    nc.vector.tensor_sub(out=o13[:, 0], in0=b_, in1=d_)
                    nc.vector.tensor_add(out=o13[:, 1], in0=b_, in1=d_)
                    nc.scalar.dma_start(out=od_d[h][c][1], in_=o13)
                    for xc in range(XC):
                        mm(ps, c, xc, "g3", h)
                    nc.vector.tensor_sub(out=c_, in0=v_, in1=ps["g3"])
                    nc.vector.tensor_add(out=o02[:, 0], in0=a_, in1=c_)
                    nc.vector.tensor_sub(out=o02[:, 1], in0=a_, in1=c_)
                    nc.scalar.dma_start(out=od_d[h][c][0], in_=o02)
    nc.compile()
    return nc


def _prep_eigens(eigens):
    """eigens (gy, gx, 4) -> packed [YC, 128, XC, 4, 128] bf16 with
    ki = (E0, E1r, E1i, E2), [x, y]-oriented, irfft scales folded in."""
    import ml_dtypes
    e = np.ascontiguousarray(np.asarray(eigens).transpose(1, 0, 2)).astype(np.float32)
    e0 = ((e[..., 0] + e[..., 2]) + (e[..., 1] + e[..., 3])) * 0.25
    e2 = ((e[..., 0] + e[..., 2]) - (e[..., 1] + e[..., 3])) * 0.25
    e1r = (e[..., 0] - e[..., 2]) * 0.5
    e1i = (e[..., 3] - e[..., 1]) * 0.5

    def chunk(m):  # [1024x, 1024y] -> [YC, 128p, XC, 128y]
        return m.reshape(XC, 128, YC, 128).transpose(2, 1, 0, 3)
    pk = np.stack([chunk(m) for m in (e0, e1r, e1i, e2)], axis=3)
    return {"e_pk": np.ascontiguousarray(pk).astype(ml_dtypes.bfloat16)}


def _prep_x(xs):
    """x shard [BS, 4096] f32 -> packed [2, 128, XC, 4, 256] bf16 with
    ki = (X0, X1r, X2, X1i)."""
    import ml_dtypes
    xb = xs.reshape(BS, GX, 4)
    x0, x1, x2, x3 = (xb[..., j] for j in range(4))
    mats = (x0 + x1 + x2 + x3, x0 - x2, x0 - x1 + x2 - x3, x3 - x1)

    def chunk(m):  # [BS, 1024x] -> [2h, 128p, XC, 256b]
        return m.T.reshape(XC, 128, 2, 256).transpose(2, 1, 0, 3)
    pk = np.stack([chunk(m) for m in mats], axis=3)
    return {"x_pk": np.ascontiguousarray(pk).astype(ml_dtypes.bfloat16)}


def _in_maps(x, eigens):
    x = np.ascontiguousarray(np.asarray(x), dtype=np.float32)
    emaps = _prep_eigens(eigens)
    return [dict(_prep_x(x[c * BS:(c + 1) * BS]), **emaps) for c in range(NCORES)]


def _assemble(results):
    # od [2h, YC, 2grp, 128y, 2pair, 256b] bf16 -> [BS, 4096] f32 per core.
    outs = []
    for r in results:
        arr = np.asarray(r["out"])
        full = np.empty((2, YC, 128, 4, 256), dtype=arr.dtype)
        full[:, :, :, 0] = arr[:, :, 0, :, 0]
        full[:, :, :, 2] = arr[:, :, 0, :, 1]
        full[:, :, :, 1] = arr[:, :, 1, :, 0]
        full[:, :, :, 3] = arr[:, :, 1, :, 1]
        outs.append(full.transpose(0, 4, 1, 2, 3).reshape(BS, OUT).astype(np.float32))
    return np.concatenate(outs, axis=0)


def kernel(x, eigens):
    from concourse.bass_utils import run_bass_kernel_spmd

    if "nc" not in _cache:
        _cache["nc"] = _build_nc()
    res = run_bass_kernel_spmd(_cache["nc"], _in_maps(x, eigens),
                               core_ids=list(range(NCORES)))
    return _assemble(res.results)


# revision 14
# speedup vs baseline: 1.0448x; 1.0448x over previous
"""Block-circulant linear (MINI_BLOCK=4) via length-4 rFFT factorization on 8 trn2 cores.

Math: out = x @ W^T where W[4y+n, 4x+j] = eigens[y, x, (n-j) mod 4].
In the length-4 DFT domain the circulant contraction factors into 5 real
matmul chains over the block-index axis gx=1024 (Gauss 3-mult for the complex
bin):
  X0 = x0+x1+x2+x3, X1 = (x0-x2) + i(x3-x1), X2 = x0-x1+x2-x3  (per block of 4)
  g1 = (X1r+X1i)E1r, g2 = X1r(E1i-E1r), g3 = X1i(E1r+E1i)
  Y0 = X0 E0, Y2 = X2 E2, Y1r = g1-g3, Y1i = g1+g2
  o0 = Y0+Y1r+Y2, o1 = Y0-Y1i-Y2, o2 = Y0-Y1r+Y2, o3 = Y0+Y1i-Y2  (scales in E)

Sharding: data-parallel over batch, 512 rows per core; E replicated.

Schedule (v2, HAM/DMA-aware):
- The PE clock-gate (HAM) keeps the array at 1.2 GHz until ~3.4us of
  sustained activity; idle gaps re-throttle it. A block of dummy matmuls on
  a zeroed SBUF tile wakes the PE right after the NEFF preamble so the array
  is warm when real data lands, and pass 0 trails the input stream at
  xc-slice granularity so the PE never takes one long stall.
- Inputs ship 4-bin-packed ([p, xc, bin, cols]) so each DMA call moves
  1-2 MB with 4-16 KB contiguous per-partition lines (14 input calls total
  on the SP ring, issued in exact consumption order). Derived bins
  (Ed=E1i-E1r, Es=E1i+E1r, X1s=X1r+X1i) are computed on device, halves
  split so the derive never gates a pass start.
- Per (128-wide y-chunk c, 256-wide batch half h) the five bins accumulate
  ps[k] = sum_xc E_k[c][:,xc,:].T @ X_k[h][:,xc,:]  -> [128 y, 256 b] PSUM
  (8 matmuls of 128x128x256 bf16 per bin), then ACT drains y0/g1, DVE
  combines the inverse DFT in bf16 and stores two [128,2,256] groups
  (n in {0,2} then {1,3}) per pass on the ACT ring so the tail store
  overlaps the drain chain. The last pass staggers bin completion
  (y0,g1,y2 xc-major, then g3, then g2) so its drains overlap its matmuls.
- PSUM: one bank per bin accumulator, bufs y0/g1:1 (freed first by the ACT
  copies), g2/y2/g3:2 = 8 banks.
Host de-interleaves [h,c,grp,y,pair,b] -> [256h+b, 4*(128c+y)+n].
"""
import numpy as np

B, IN, OUT, BLK = 4096, 4096, 4096, 4
GX, GY = IN // BLK, OUT // BLK        # 1024, 1024
NCORES = 8
BS = B // NCORES                      # 512 batch rows per core
XC = GX // 128                        # 8 x-chunks (contraction)
YC = GY // 128                        # 8 y-chunks (output)
BINS = ("y0", "g2", "y2", "g3", "g1")
PSUM_BUFS = {"y0": 1, "g1": 1, "g2": 2, "y2": 2, "g3": 2}
# PE warm-up matmuls: span preamble-end (~7.6us) to first X half landing
# (~14.2us): ~16 cold at 213ns (HAM un-throttles after ~3.4us) + ~28 warm.
NDUMMY = 44

_cache = {}


def _build_nc():
    from concourse import bacc
    import concourse.mybir as mybir
    from concourse.tile import TileContext

    f32 = mybir.dt.float32
    bf16 = mybir.dt.bfloat16

    nc = bacc.Bacc("TRN2", target_bir_lowering=False, debug=False,
                   enable_asserts=False, num_devices=NCORES)
    # E packed per chunk: [yc, p, xc, ki, y] with ki = (E0, E1r, E1i, E2).
    # 8KB contiguous per partition per chunk -> one call per chunk.
    e_pk = nc.dram_tensor("e_pk", [YC, 128, XC, 4, 128], bf16,
                          kind="ExternalInput")
    # X packed per half: [h, p, xc, ki, b] with ki = (X0, X1r, X2, X1i).
    # 16KB lines; half 0 ships in 2-xc slices for pass-0 trailing.
    x_pk = nc.dram_tensor("x_pk", [2, 128, XC, 4, 256], bf16,
                          kind="ExternalInput")
    # out [h, yc, grp, y, pair, b]: grp0 pairs (n0,n2), grp1 pairs (n1,n3).
    od_d = nc.dram_tensor("out", [2, YC, 2, 128, 2, 256], bf16,
                          kind="ExternalOutput")

    with TileContext(nc) as tc:
        with (
            tc.tile_pool(name="xm", bufs=1) as xp,
            tc.tile_pool(name="es", bufs=1) as ep,
            tc.tile_pool(name="tv", bufs=2) as tvp,
            tc.tile_pool(name="abcd", bufs=3) as ab,
            tc.tile_pool(name="otp", bufs=8) as op_,
            tc.tile_pool(name="ps", bufs=1, space="PSUM") as mps,
        ):
            # Persistent SBUF residency: X 32KB/part + 8KB derived,
            # E 64KB/part + 32KB derived.
            xm = [xp.tile([128, XC, 4, 256], bf16, tag=f"xm{h}", name=f"xm{h}")
                  for h in range(2)]
            xg = [xp.tile([128, XC, 256], bf16, tag=f"xg{h}", name=f"xg{h}")
                  for h in range(2)]
            es_pk = [ep.tile([128, XC, 4, 128], bf16, tag=f"es{c}",
                             name=f"es{c}") for c in range(YC)]
            ed_t = [ep.tile([128, XC, 128], bf16, tag=f"ed{c}", name=f"ed{c}")
                    for c in range(YC)]
            es_t = [ep.tile([128, XC, 128], bf16, tag=f"eds{c}",
                            name=f"eds{c}") for c in range(YC)]
            jk = xp.tile([128, 384], bf16, tag="jk", name="jk")

            # Input DMA on the SP ring in exact consumption order; the queue
            # drains FIFO so issue order == arrival order. The 16 SDMA
            # engines cap at ~26 GB/s each, so big descriptors (8-16KB) and
            # few call boundaries maximize the ~420 GB/s aggregate.
            nc.sync.dma_start(out=es_pk[0], in_=e_pk[0])
            for q in range(2):
                s = slice(4 * q, 4 * q + 4)
                nc.sync.dma_start(out=xm[0][:, s], in_=x_pk[0][:, s])
            nc.sync.dma_start(out=es_pk[1][:, 0:4], in_=e_pk[1][:, 0:4])
            nc.sync.dma_start(out=es_pk[1][:, 4:8], in_=e_pk[1][:, 4:8])
            for c in range(2, YC):
                nc.sync.dma_start(out=es_pk[c], in_=e_pk[c])
            nc.sync.dma_start(out=xm[1], in_=x_pk[1])

            # PE warm-up: zero a junk tile, then back-to-back dummy matmuls
            # into pass-0's y0 bank (its real group start=True resets it).
            nc.gpsimd.memset(jk, 0)

            def make_ps():
                return {k: mps.tile([128, 256], f32, tag=f"ps_{k}",
                                    name=f"ps_{k}", bufs=PSUM_BUFS[k])
                        for k in BINS}

            ps0 = make_ps()
            for _ in range(NDUMMY):
                nc.tensor.matmul(ps0["y0"], jk[:, 0:128], jk[:, 128:384],
                                 start=True, stop=True, skip_group_check=True)

            # Derived bins; chunk-0 and chunk-1 derives split by xc-half so
            # they trail the (split) DMA calls without gating pass starts.
            def derive_E(c, lo=0, hi=XC):
                s = slice(lo, hi)
                nc.vector.tensor_sub(out=ed_t[c][:, s], in0=es_pk[c][:, s, 2],
                                     in1=es_pk[c][:, s, 1])
                nc.vector.tensor_add(out=es_t[c][:, s], in0=es_pk[c][:, s, 2],
                                     in1=es_pk[c][:, s, 1])

            def derive_g1(h, lo=0, hi=XC):
                s = slice(lo, hi)
                nc.vector.tensor_add(out=xg[h][:, s], in0=xm[h][:, s, 1],
                                     in1=xm[h][:, s, 3])

            derive_E(0, 0, 4)
            derive_E(0, 4, 8)
            derive_g1(0, 0, 4)
            derive_g1(0, 4, 8)

            # bin -> (stationary source, moving ki slot); g2/g3 read derived
            # E tiles, g1 reads the derived X tile.
            def stat(k, c, xc):
                if k == "y0":
                    return es_pk[c][:, xc, 0]
                if k == "g1":
                    return es_pk[c][:, xc, 1]
                if k == "y2":
                    return es_pk[c][:, xc, 3]
                if k == "g2":
                    return ed_t[c][:, xc]
                return es_t[c][:, xc]

            def mov(k, h, xc):
                ki = {"y0": 0, "g2": 1, "y2": 2, "g3": 3}.get(k)
                return xg[h][:, xc] if ki is None else xm[h][:, xc, ki]

            def mm(ps, c, xc, k, h):
                nc.tensor.matmul(ps[k], stat(k, c, xc), mov(k, h, xc),
                                 start=xc == 0, stop=xc == XC - 1,
                                 skip_group_check=True)

            def drains(ps, c, h):
                # Inverse DFT: ACT drains g1/y0 (frees their single banks
                # fast); DVE reads one PSUM operand per op, combines in bf16.
                # Two stores per pass so the last store never serializes the
                # whole drain chain.
                v_ = tvp.tile([128, 256], f32, tag="v", name="v_")
                t_ = tvp.tile([128, 256], f32, tag="t", name="t_")
                a_ = ab.tile([128, 256], bf16, tag="a", name="a_")
                b_ = ab.tile([128, 256], bf16, tag="b", name="b_")
                c_ = ab.tile([128, 256], bf16, tag="c", name="c_")
                d_ = ab.tile([128, 256], bf16, tag="d", name="d_")
                o02 = op_.tile([128, 2, 256], bf16, tag="o02", name="o02")
                o13 = op_.tile([128, 2, 256], bf16, tag="o13", name="o13")
                nc.scalar.copy(out=t_, in_=ps["y0"])              # frees y0
                nc.scalar.copy(out=v_, in_=ps["g1"])              # frees g1
                nc.vector.tensor_add(out=a_, in0=t_, in1=ps["y2"])  # Y0+Y2
                nc.vector.tensor_sub(out=b_, in0=t_, in1=ps["y2"])  # Y0-Y2
                nc.vector.tensor_sub(out=c_, in0=v_, in1=ps["g3"])  # Y1r
                nc.vector.tensor_add(out=o02[:, 0], in0=a_, in1=c_)
                nc.vector.tensor_sub(out=o02[:, 1], in0=a_, in1=c_)
                nc.sync.dma_start(out=od_d[h][c][0], in_=o02)
                nc.vector.tensor_add(out=d_, in0=v_, in1=ps["g2"])  # Y1i
                nc.vector.tensor_sub(out=o13[:, 0], in0=b_, in1=d_)
                nc.vector.tensor_add(out=o13[:, 1], in0=b_, in1=d_)
                nc.sync.dma_start(out=od_d[h][c][1], in_=o13)

            for h in range(2):
                for c in range(YC):
                    ps = ps0 if (h == 0 and c == 0) else make_ps()
                    if not (h == 1 and c == YC - 1):
                        for xc in range(XC):
                            for k in BINS:
                                mm(ps, c, xc, k, h)
                        if h == 0 and c == 0:
                            # Pass-1 needs ed/es for chunk 1 right away;
                            # the first half lands before the DVE reaches it.
                            derive_E(1, 0, 4)
                        drains(ps, c, h)
                        # Derives AFTER the drains: a late chunk landing can
                        # then never head-block the drain chain in the DVE
                        # FIFO (which backs up into PSUM-bank PE stalls).
                        if h == 0 and c == 0:
                            derive_E(1, 4, 8)
                        elif h == 0 and c < YC - 1:
                            derive_E(c + 1)
                        if h == 0 and c == 5:
                            # X half-1 lands ~38us, well before pass 8.
                            derive_g1(1, 0, 4)
                            derive_g1(1, 4, 8)
                        continue
                    # Tail pass: stagger bin completion so the drains and
                    # first store overlap this pass's own matmuls.
                    for xc in range(XC):
                        for k in ("y0", "g1", "y2"):
                            mm(ps, c, xc, k, h)
                    v_ = tvp.tile([128, 256], f32, tag="v", name="v_")
                    t_ = tvp.tile([128, 256], f32, tag="t", name="t_")
                    a_ = ab.tile([128, 256], bf16, tag="a", name="a_")
                    b_ = ab.tile([128, 256], bf16, tag="b", name="b_")
                    c_ = ab.tile([128, 256], bf16, tag="c", name="c_")
                    d_ = ab.tile([128, 256], bf16, tag="d", name="d_")
                    o02 = op_.tile([128, 2, 256], bf16, tag="o02", name="o02")
                    o13 = op_.tile([128, 2, 256], bf16, tag="o13", name="o13")
                    nc.scalar.copy(out=t_, in_=ps["y0"])
                    nc.scalar.copy(out=v_, in_=ps["g1"])
                    nc.vector.tensor_add(out=a_, in0=t_, in1=ps["y2"])
                    nc.vector.tensor_sub(out=b_, in0=t_, in1=ps["y2"])
                    for xc in range(XC):
                        mm(ps, c, xc, "g2", h)
                    nc.vector.tensor_add(out=d_, in0=v_, in1=ps["g2"])
                    nc.vector.tensor_sub(out=o13[:, 0], in0=b_, in1=d_)
                    nc.vector.tensor_add(out=o13[:, 1], in0=b_, in1=d_)
                    nc.sync.dma_start(out=od_d[h][c][1], in_=o13)
                    for xc in range(XC):
                        mm(ps, c, xc, "g3", h)
                    nc.vector.tensor_sub(out=c_, in0=v_, in1=ps["g3"])
                    nc.vector.tensor_add(out=o02[:, 0], in0=a_, in1=c_)
                    nc.vector.tensor_sub(out=o02[:, 1], in0=a_, in1=c_)
                    nc.sync.dma_start(out=od_d[h][c][0], in_=o02)
    nc.compile()
    return nc


def _prep_eigens(eigens):
    """eigens (gy, gx, 4) -> packed [YC, 128, XC, 4, 128] bf16 with
    ki = (E0, E1r, E1i, E2), [x, y]-oriented, irfft scales folded in."""
    import ml_dtypes
    e = np.ascontiguousarray(np.asarray(eigens).transpose(1, 0, 2)).astype(np.float32)
    e0 = ((e[..., 0] + e[..., 2]) + (e[..., 1] + e[..., 3])) * 0.25
    e2 = ((e[..., 0] + e[..., 2]) - (e[..., 1] + e[..., 3])) * 0.25
    e1r = (e[..., 0] - e[..., 2]) * 0.5
    e1i = (e[..., 3] - e[..., 1]) * 0.5

    def chunk(m):  # [1024x, 1024y] -> [YC, 128p, XC, 128y]
        return m.reshape(XC, 128, YC, 128).transpose(2, 1, 0, 3)
    pk = np.stack([chunk(m) for m in (e0, e1r, e1i, e2)], axis=3)
    return {"e_pk": np.ascontiguousarray(pk).astype(ml_dtypes.bfloat16)}


def _prep_x(xs):
    """x shard [BS, 4096] f32 -> packed [2, 128, XC, 4, 256] bf16 with
    ki = (X0, X1r, X2, X1i)."""
    import ml_dtypes
    xb = xs.reshape(BS, GX, 4)
    x0, x1, x2, x3 = (xb[..., j] for j in range(4))
    mats = (x0 + x1 + x2 + x3, x0 - x2, x0 - x1 + x2 - x3, x3 - x1)

    def chunk(m):  # [BS, 1024x] -> [2h, 128p, XC, 256b]
        return m.T.reshape(XC, 128, 2, 256).transpose(2, 1, 0, 3)
    pk = np.stack([chunk(m) for m in mats], axis=3)
    return {"x_pk": np.ascontiguousarray(pk).astype(ml_dtypes.bfloat16)}


def _in_maps(x, eigens):
    x = np.ascontiguousarray(np.asarray(x), dtype=np.float32)
    emaps = _prep_eigens(eigens)
    return [dict(_prep_x(x[c * BS:(c + 1) * BS]), **emaps) for c in range(NCORES)]


def _assemble(results):
    # od [2h, YC, 2grp, 128y, 2pair, 256b] bf16 -> [BS, 4096] f32 per core.
    outs = []
    for r in results:
        arr = np.asarray(r["out"])
        full = np.empty((2, YC, 128, 4, 256), dtype=arr.dtype)
        full[:, :, :, 0] = arr[:, :, 0, :, 0]
        full[:, :, :, 2] = arr[:, :, 0, :, 1]
        full[:, :, :, 1] = arr[:, :, 1, :, 0]
        full[:, :, :, 3] = arr[:, :, 1, :, 1]
        outs.append(full.transpose(0, 4, 1, 2, 3).reshape(BS, OUT).astype(np.float32))
    return np.concatenate(outs, axis=0)


def kernel(x, eigens):
    from concourse.bass_utils import run_bass_kernel_spmd

    if "nc" not in _cache:
        _cache["nc"] = _build_nc()
    res = run_bass_kernel_spmd(_cache["nc"], _in_maps(x, eigens),
                               core_ids=list(range(NCORES)))
    return _assemble(res.results)
